# revision 1
# baseline (speedup 1.0000x reference)
"""Trainium2 Bass kernel for nn_CollaborativeRNNModel.

Model (per reference):
  per step t (T=100), batch b (B=64), hidden H=128:
    g_u = h @ gate_ku[uid,:,128:256] + gate_bias[128:] + gate_ki[iid,128:]
    u   = sigmoid(g_u)                       (r-half is computed but unused)
    c   = tanh(h @ cand_ku[uid] + cand_bias + cand_ki[iid])
    h'  = u*h + (1-u)*c
  logits = states[B*T, H] @ ws[H, 20001]

Sharding: data-parallel over batch, 8 rows per core, full tables on
every core, no collectives.  Per-core kernel keeps h transposed
[H=128 partitions, 8 cols]; per-user weights are gathered per step with
register-offset HWDGE DMAs from a combined [gate_u|cand] table; the
final logits matmul (float32r) is interleaved with the recurrence.
"""

import numpy as np

import concourse.bass as bass
import concourse.bacc as bacc
import concourse.tile as tile
import concourse.mybir as mybir
import concourse.bass_utils as bass_utils
from concourse.masks import make_identity

H = 128
U = 5000
I = 20000
B = 64
T = 100
N_CORES = 8
BPC = B // N_CORES          # batch rows per core = 8
V = I + 1                   # vocab/items = 20001
NI = BPC * T                # rows per core = 800
VCHUNK = 512
F32 = mybir.dt.float32
F32R = mybir.dt.float32r
I32 = mybir.dt.int32


def build_nc(t_steps=T, use_f32r=False):
    """Build and compile the per-core Bass program (SPMD, same on all cores)."""
    ni = BPC * t_steps
    n_mtiles = (ni + 127) // 128
    WDT = F32R if use_f32r else F32   # dtype for matmul operands

    nc = bacc.Bacc("TRN2", target_bir_lowering=False, debug=False,
                   enable_asserts=False, num_devices=N_CORES)

    # DRAM inputs (per core)
    # uidx[0, i] = uid_i * H  (row offset into the combined [U*H, 2H] table)
    uids_d = nc.dram_tensor("uidx", [1, ni], I32, kind="ExternalInput")
    # iid_arr[p, cb] = item id for flat index i = cb*128 + p (i = t*8+b), 0-padded
    n_cb = (ni + 127) // 128
    iids_d = nc.dram_tensor("iids", [128, n_cb], I32, kind="ExternalInput")
    h0t_d = nc.dram_tensor("h0t", [H, BPC], WDT, kind="ExternalInput")
    wcomb_d = nc.dram_tensor("wcomb", [(U + 1) * H, 2 * H], F32, kind="ExternalInput")
    kicomb_d = nc.dram_tensor("kicomb", [V, 2 * H], F32, kind="ExternalInput")
    bias_u_d = nc.dram_tensor("bias_u", [H, 1], F32, kind="ExternalInput")
    bias_c_d = nc.dram_tensor("bias_c", [H, 1], F32, kind="ExternalInput")
    ws_d = nc.dram_tensor("ws", [H, V], WDT, kind="ExternalInput")
    out_d = nc.dram_tensor("logits", [ni, V], F32, kind="ExternalOutput")

    with tile.TileContext(nc) as tc:
        with (
            tc.tile_pool(name="const", bufs=1) as cpool,
            tc.tile_pool(name="big", bufs=1) as bpool,
            tc.tile_pool(name="w", bufs=6) as wpool,
            tc.tile_pool(name="sm", bufs=3) as spool,
            tc.tile_pool(name="stage", bufs=4) as stpool,
            tc.tile_pool(name="prec", bufs=2, space="PSUM") as prec,
            tc.tile_pool(name="pfin", bufs=3, space="PSUM") as pfin,
            tc.tile_pool(name="ptr", bufs=1, space="PSUM") as ptr,
        ):
            # ---- constants / one-time loads ----
            ident = cpool.tile([128, 128], F32, tag="ident")
            make_identity(nc, ident[:])

            uid_sb = cpool.tile([1, ni], I32, tag="uid")
            nc.gpsimd.dma_start(uid_sb[:], uids_d.ap())
            iid_sb = cpool.tile([128, n_cb], I32, tag="iid")
            nc.gpsimd.dma_start(iid_sb[:], iids_d.ap())
            bias_u = cpool.tile([H, 1], F32, tag="bu")
            nc.gpsimd.dma_start(bias_u[:], bias_u_d.ap())
            bias_c = cpool.tile([H, 1], F32, tag="bc")
            nc.gpsimd.dma_start(bias_c[:], bias_c_d.ap())

            # states^T: col 8*0..8 = h0, col 8 + (t*8+b) = state after step t, col b
            statesT = bpool.tile([H, 8 * (t_steps + 1)], WDT, tag="statesT")
            nc.gpsimd.dma_start(statesT[:, 0:BPC], h0t_d.ap())

            # ws resident in SBUF
            ws_sb = bpool.tile([H, V], WDT, tag="ws")
            nc.sync.dma_start(ws_sb[:], ws_d.ap())

            # ---- item-embedding gather + transpose (one-time) ----
            # G[p, cb*256 + k] = kicomb[iid_arr[p, cb], k]
            G = bpool.tile([128, n_cb * 2 * H], F32, tag="G")
            for cb in range(n_cb):
                nc.gpsimd.indirect_dma_start(
                    out=G[:, cb * 256:(cb + 1) * 256],
                    out_offset=None,
                    in_=kicomb_d.ap(),
                    in_offset=bass.IndirectOffsetOnAxis(ap=iid_sb[:, cb:cb + 1], axis=0),
                )
            # ukiT_b[k, i] = gate_ki_u[iid_i, k] + bias_u[k];  ckiT_b likewise
            ukiT = bpool.tile([128, n_cb * 128], F32, tag="ukiT")
            ckiT = bpool.tile([128, n_cb * 128], F32, tag="ckiT")
            for cb in range(n_cb):
                for half, (dst, bias_t) in enumerate(((ukiT, bias_u), (ckiT, bias_c))):
                    tp = ptr.tile([128, 128], F32, tag="tps")
                    nc.tensor.transpose(
                        tp[:], G[:, cb * 256 + half * 128: cb * 256 + (half + 1) * 128],
                        ident[:])
                    nc.vector.tensor_scalar(
                        out=dst[:, cb * 128:(cb + 1) * 128], in0=tp[:],
                        scalar1=bias_t[:], scalar2=None, op0=mybir.AluOpType.add)

            # ---- recurrence + interleaved final matmul ----
            STG = 4 * VCHUNK  # output staging width (8KB/partition DMAs)

            def emit_mtile(m):
                lo = m * 128
                mw = min(128, ni - lo)
                lhs = statesT[:, 8 + lo: 8 + lo + mw]
                for cg in range(0, V, STG):
                    gw = min(STG, V - cg)
                    st = stpool.tile([128, STG], F32, tag="st")
                    for ci in range(cg, cg + gw, VCHUNK):
                        cw = min(VCHUNK, cg + gw - ci)
                        ps = pfin.tile([128, VCHUNK], F32, tag="fps")
                        nc.tensor.matmul(ps[:mw, :cw], lhsT=lhs,
                                         rhs=ws_sb[:, ci:ci + cw],
                                         start=True, stop=True)
                        nc.vector.tensor_copy(st[:mw, ci - cg:ci - cg + cw],
                                              ps[:mw, :cw])
                    nc.gpsimd.dma_start(out_d.ap()[lo:lo + mw, cg:cg + gw],
                                        st[:mw, :gw])

            # gather issue split across the three DMA-capable engines; one
            # batched multi-register load per engine per step
            GROUPS = ((nc.sync, 0, 3), (nc.scalar, 3, 2), (nc.gpsimd, 5, 3))
            m_emitted = 0
            gchain = {}
            for t in range(t_steps):
                # wt[h, b*256 + k] = wcomb[uid[b,t]*H + h, k]
                wt = wpool.tile([128, BPC * 2 * H], F32, tag="wt")
                for eng, b0, nb in GROUPS:
                    regs = [eng.alloc_register(f"g_{t}_{b0 + j}")
                            for j in range(nb)]
                    ld = eng.reg_load(
                        regs, uid_sb[0:1, t * BPC + b0: t * BPC + b0 + nb])
                    key = id(eng)
                    hist = gchain.setdefault(key, [])
                    if len(hist) >= 2:
                        tile.add_dep_helper(hist[-2], ld.ins, False, "gorder")
                    for j in range(nb):
                        b = b0 + j
                        row = eng.snap(regs[j], donate=True,
                                       min_val=0, max_val=U * H)
                        dma = eng.dma_start(wt[:, b * 256:(b + 1) * 256],
                                            wcomb_d.ap()[bass.ds(row, H), :])
                    hist.append(dma.ins)

                ps_u = prec.tile([128, BPC], F32, tag="psu")
                ps_c = prec.tile([128, BPC], F32, tag="psc")
                h_prev = statesT[:, t * 8: t * 8 + BPC]
                h_prev_mm = h_prev.bitcast(F32) if use_f32r else h_prev
                for b in range(BPC):
                    nc.tensor.matmul(ps_u[:, b:b + 1],
                                     lhsT=wt[:, b * 256: b * 256 + 128],
                                     rhs=h_prev_mm[:, b:b + 1], start=True, stop=True)
                    nc.tensor.matmul(ps_c[:, b:b + 1],
                                     lhsT=wt[:, b * 256 + 128: (b + 1) * 256],
                                     rhs=h_prev_mm[:, b:b + 1], start=True, stop=True)

                cols = slice(t * 8, t * 8 + BPC)
                tu = spool.tile([128, BPC], F32, tag="tu")
                nc.vector.tensor_add(tu[:], ps_u[:], ukiT[:, cols])
                uu = spool.tile([128, BPC], F32, tag="uu")
                nc.scalar.activation(uu[:], tu[:], mybir.ActivationFunctionType.Sigmoid)
                tcn = spool.tile([128, BPC], F32, tag="tc")
                nc.vector.tensor_add(tcn[:], ps_c[:], ckiT[:, cols])
                cc = spool.tile([128, BPC], F32, tag="cc")
                nc.scalar.activation(cc[:], tcn[:], mybir.ActivationFunctionType.Tanh)
                dd = spool.tile([128, BPC], F32, tag="dd")
                nc.vector.tensor_sub(dd[:], h_prev, cc[:])
                ee = spool.tile([128, BPC], F32, tag="ee")
                nc.vector.tensor_mul(ee[:], uu[:], dd[:])
                nc.vector.tensor_add(statesT[:, (t + 1) * 8:(t + 1) * 8 + BPC],
                                     cc[:], ee[:])

                # emit final-matmul tiles as soon as their states are complete
                while m_emitted < n_mtiles and (m_emitted + 1) * 128 <= (t + 1) * 8:
                    emit_mtile(m_emitted)
                    m_emitted += 1
            while m_emitted < n_mtiles:
                emit_mtile(m_emitted)
                m_emitted += 1

    nc.compile()
    return nc


def prep_inputs(user_ids, item_ids, h0, gate_ku, gate_ki, gate_bias,
                cand_ku, cand_ki, cand_bias, ws, t_steps=T):
    """Host-side sharding/arrangement -> per-core in_maps."""
    ni = BPC * t_steps
    n_cb = (ni + 127) // 128
    wcomb = np.concatenate([gate_ku[:, :, H:], cand_ku], axis=2)
    wcomb = np.ascontiguousarray(wcomb, np.float32).reshape((U + 1) * H, 2 * H)
    kicomb = np.ascontiguousarray(
        np.concatenate([gate_ki[:, H:], cand_ki], axis=1), np.float32)
    bias_u = np.ascontiguousarray(gate_bias[H:].reshape(H, 1), np.float32)
    bias_c = np.ascontiguousarray(cand_bias.reshape(H, 1), np.float32)
    ws_c = np.ascontiguousarray(ws, np.float32)

    in_maps = []
    for c in range(N_CORES):
        rows = slice(c * BPC, (c + 1) * BPC)
        # uidx[0, i] = uid_i*H : row offsets into the combined table, i = t*8+b
        uid_flat = np.ascontiguousarray(
            user_ids[rows, :t_steps], np.int32).T.reshape(-1)  # [ni]
        idxmat = (uid_flat * H).reshape(1, ni)
        iids_flat = np.ascontiguousarray(
            item_ids[rows, :t_steps], np.int32).T.reshape(-1)  # i = t*8+b
        iid_arr = np.zeros(n_cb * 128, np.int32)
        iid_arr[:ni] = iids_flat
        iid_arr = np.ascontiguousarray(iid_arr.reshape(n_cb, 128).T)
        h0t = np.ascontiguousarray(h0[rows].T, np.float32)
        in_maps.append({
            "uidx": np.ascontiguousarray(idxmat), "iids": iid_arr, "h0t": h0t,
            "wcomb": wcomb, "kicomb": kicomb,
            "bias_u": bias_u, "bias_c": bias_c, "ws": ws_c,
        })
    return in_maps


def assemble_output(results, t_steps=T):
    ni = BPC * t_steps
    out = np.empty((B * t_steps, V), np.float32)
    for c in range(N_CORES):
        blk = results[c]["logits"]  # [ni, V], rows i = t*8+b
        out[c * ni:(c + 1) * ni] = (
            blk.reshape(t_steps, BPC, V).transpose(1, 0, 2).reshape(ni, V))
    return out


_NC_CACHE = {}
USE_F32R = False


def _get_nc(t_steps=T, use_f32r=None):
    if use_f32r is None:
        use_f32r = USE_F32R
    key = (t_steps, use_f32r)
    if key not in _NC_CACHE:
        _NC_CACHE[key] = build_nc(t_steps, use_f32r=use_f32r)
    return _NC_CACHE[key]


def kernel(user_ids, item_ids, h0, gate_ku, gate_ki, gate_bias,
           cand_ku, cand_ki, cand_bias, ws, trace=False):
    nc = _get_nc(T)
    in_maps = prep_inputs(np.asarray(user_ids), np.asarray(item_ids),
                          np.asarray(h0), np.asarray(gate_ku),
                          np.asarray(gate_ki), np.asarray(gate_bias),
                          np.asarray(cand_ku), np.asarray(cand_ki),
                          np.asarray(cand_bias), np.asarray(ws))
    res = bass_utils.run_bass_kernel_spmd(
        nc, in_maps, core_ids=list(range(N_CORES)), trace=trace)
    out = assemble_output(res.results)
    if trace:
        kernel.last_result = res
    return out



# revision 2
# speedup vs baseline: 2.6295x; 2.6295x over previous
"""Trainium2 Bass kernel for nn_CollaborativeRNNModel.

Model (per reference):
  per step t (T=100), batch b (B=64), hidden H=128:
    g_u = h @ gate_ku[uid,:,128:256] + gate_bias[128:] + gate_ki[iid,128:]
    u   = sigmoid(g_u)                       (r-half is computed but unused)
    c   = tanh(h @ cand_ku[uid] + cand_bias + cand_ki[iid])
    h'  = u*h + (1-u)*c
  logits = states[B*T, H] @ ws[H, 20001]

Sharding: data-parallel over batch, 8 rows per core, full tables on
every core, no collectives.  Per-core kernel keeps h transposed
[H=128 partitions, 8 cols]; per-user weights are gathered per step with
register-offset HWDGE DMAs from a combined [gate_u|cand] table; the
final logits matmul is interleaved with the recurrence.

All matmul operands (gathered per-user weights, states, ws) and the
logits output are bf16: fp32 matmuls cost 4 PE-cycles/row and two
LDWEIGHTS+MATMUL pairs each, while bf16 costs 1 cycle/row with
fast-weight-load — and bf16 tables/outputs halve the dominant HBM
traffic (gathers + logit stores).  Accumulation stays fp32 in PSUM and
the h-update runs in fp32 on DVE, so the recurrence error stays small
(sigmoid gate contraction bounds bf16 noise; measured ~2e-3 rel).
"""

import numpy as np
import ml_dtypes

import concourse.bass as bass
import concourse.bacc as bacc
import concourse.tile as tile
import concourse.mybir as mybir
import concourse.bass_utils as bass_utils
from concourse.masks import make_identity

H = 128
U = 5000
I = 20000
B = 64
T = 100
N_CORES = 8
BPC = B // N_CORES          # batch rows per core = 8
V = I + 1                   # vocab/items = 20001
NI = BPC * T                # rows per core = 800
VCHUNK = 512
F32 = mybir.dt.float32
BF16 = mybir.dt.bfloat16
I32 = mybir.dt.int32
NP_BF16 = ml_dtypes.bfloat16


def build_nc(t_steps=T):
    """Build and compile the per-core Bass program (SPMD, same on all cores)."""
    ni = BPC * t_steps
    n_mtiles = (ni + 127) // 128

    nc = bacc.Bacc("TRN2", target_bir_lowering=False, debug=False,
                   enable_asserts=False, num_devices=N_CORES)

    # DRAM inputs (per core)
    # uidx[0, i] = uid_i * H  (row offset into the combined [U*H, 2H] table)
    uids_d = nc.dram_tensor("uidx", [1, ni], I32, kind="ExternalInput")
    # iid_arr[p, cb] = item id for flat index i = cb*128 + p (i = t*8+b), 0-padded
    n_cb = (ni + 127) // 128
    iids_d = nc.dram_tensor("iids", [128, n_cb], I32, kind="ExternalInput")
    h0t_d = nc.dram_tensor("h0t", [H, BPC], BF16, kind="ExternalInput")
    wcomb_d = nc.dram_tensor("wcomb", [(U + 1) * H, 2 * H], BF16, kind="ExternalInput")
    kicomb_d = nc.dram_tensor("kicomb", [V, 2 * H], F32, kind="ExternalInput")
    bias_u_d = nc.dram_tensor("bias_u", [H, 1], F32, kind="ExternalInput")
    bias_c_d = nc.dram_tensor("bias_c", [H, 1], F32, kind="ExternalInput")
    ws_d = nc.dram_tensor("ws", [H, V], BF16, kind="ExternalInput")
    out_d = nc.dram_tensor("logits", [ni, V], BF16, kind="ExternalOutput")

    with tile.TileContext(nc) as tc:
        with (
            tc.tile_pool(name="const", bufs=1) as cpool,
            tc.tile_pool(name="big", bufs=1) as bpool,
            tc.tile_pool(name="w", bufs=6) as wpool,
            tc.tile_pool(name="sm", bufs=3) as spool,
            tc.tile_pool(name="stage", bufs=4) as stpool,
            tc.tile_pool(name="prec", bufs=2, space="PSUM") as prec,
            tc.tile_pool(name="pfin", bufs=3, space="PSUM") as pfin,
            tc.tile_pool(name="ptr", bufs=1, space="PSUM") as ptr,
        ):
            # ---- constants / one-time loads ----
            ident = cpool.tile([128, 128], F32, tag="ident")
            make_identity(nc, ident[:])

            uid_sb = cpool.tile([1, ni], I32, tag="uid")
            nc.gpsimd.dma_start(uid_sb[:], uids_d.ap())
            iid_sb = cpool.tile([128, n_cb], I32, tag="iid")
            nc.gpsimd.dma_start(iid_sb[:], iids_d.ap())
            bias_u = cpool.tile([H, 1], F32, tag="bu")
            nc.gpsimd.dma_start(bias_u[:], bias_u_d.ap())
            bias_c = cpool.tile([H, 1], F32, tag="bc")
            nc.gpsimd.dma_start(bias_c[:], bias_c_d.ap())

            # states^T: col 8*0..8 = h0, col 8 + (t*8+b) = state after step t, col b
            statesT = bpool.tile([H, 8 * (t_steps + 1)], BF16, tag="statesT")
            nc.gpsimd.dma_start(statesT[:, 0:BPC], h0t_d.ap())

            # ws resident in SBUF
            ws_sb = bpool.tile([H, V], BF16, tag="ws")
            nc.sync.dma_start(ws_sb[:], ws_d.ap())

            # ---- item-embedding gather + transpose (one-time) ----
            # G[p, cb*256 + k] = kicomb[iid_arr[p, cb], k]
            G = bpool.tile([128, n_cb * 2 * H], F32, tag="G")
            for cb in range(n_cb):
                nc.gpsimd.indirect_dma_start(
                    out=G[:, cb * 256:(cb + 1) * 256],
                    out_offset=None,
                    in_=kicomb_d.ap(),
                    in_offset=bass.IndirectOffsetOnAxis(ap=iid_sb[:, cb:cb + 1], axis=0),
                )
            # ukiT_b[k, i] = gate_ki_u[iid_i, k] + bias_u[k];  ckiT_b likewise
            ukiT = bpool.tile([128, n_cb * 128], F32, tag="ukiT")
            ckiT = bpool.tile([128, n_cb * 128], F32, tag="ckiT")
            for cb in range(n_cb):
                for half, (dst, bias_t) in enumerate(((ukiT, bias_u), (ckiT, bias_c))):
                    tp = ptr.tile([128, 128], F32, tag="tps")
                    nc.tensor.transpose(
                        tp[:], G[:, cb * 256 + half * 128: cb * 256 + (half + 1) * 128],
                        ident[:])
                    nc.vector.tensor_scalar(
                        out=dst[:, cb * 128:(cb + 1) * 128], in0=tp[:],
                        scalar1=bias_t[:], scalar2=None, op0=mybir.AluOpType.add)

            # ---- recurrence + interleaved final matmul ----
            STG = 4 * VCHUNK  # output staging width (4KB/partition bf16 DMAs)
            st_engines = (nc.sync, nc.scalar, nc.gpsimd)
            st_rr = [0]

            def emit_mtile(m):
                lo = m * 128
                mw = min(128, ni - lo)
                lhs = statesT[:, 8 + lo: 8 + lo + mw]
                for cg in range(0, V, STG):
                    gw = min(STG, V - cg)
                    st = stpool.tile([128, STG], BF16, tag="st")
                    for ci in range(cg, cg + gw, VCHUNK):
                        cw = min(VCHUNK, cg + gw - ci)
                        ps = pfin.tile([128, VCHUNK], F32, tag="fps")
                        nc.tensor.matmul(ps[:mw, :cw], lhsT=lhs,
                                         rhs=ws_sb[:, ci:ci + cw],
                                         start=True, stop=True)
                        nc.vector.tensor_copy(st[:mw, ci - cg:ci - cg + cw],
                                              ps[:mw, :cw])
                    eng = st_engines[st_rr[0] % len(st_engines)]
                    st_rr[0] += 1
                    eng.dma_start(out_d.ap()[lo:lo + mw, cg:cg + gw],
                                  st[:mw, :gw])

            # gather issue split across the three DMA-capable engines; one
            # batched multi-register load per engine per step
            GROUPS = ((nc.sync, 0, 3), (nc.scalar, 3, 2), (nc.gpsimd, 5, 3))
            m_emitted = 0
            gchain = {}
            for t in range(t_steps):
                # wt[h, b*256 + k] = wcomb[uid[b,t]*H + h, k]
                wt = wpool.tile([128, BPC * 2 * H], BF16, tag="wt")
                for eng, b0, nb in GROUPS:
                    regs = [eng.alloc_register(f"g_{t}_{b0 + j}")
                            for j in range(nb)]
                    ld = eng.reg_load(
                        regs, uid_sb[0:1, t * BPC + b0: t * BPC + b0 + nb])
                    key = id(eng)
                    hist = gchain.setdefault(key, [])
                    if len(hist) >= 2:
                        tile.add_dep_helper(hist[-2], ld.ins, False, "gorder")
                    for j in range(nb):
                        b = b0 + j
                        row = eng.snap(regs[j], donate=True,
                                       min_val=0, max_val=U * H)
                        dma = eng.dma_start(wt[:, b * 256:(b + 1) * 256],
                                            wcomb_d.ap()[bass.ds(row, H), :])
                    hist.append(dma.ins)

                ps_u = prec.tile([128, BPC], F32, tag="psu")
                ps_c = prec.tile([128, BPC], F32, tag="psc")
                h_prev = statesT[:, t * 8: t * 8 + BPC]
                for b in range(BPC):
                    nc.tensor.matmul(ps_u[:, b:b + 1],
                                     lhsT=wt[:, b * 256: b * 256 + 128],
                                     rhs=h_prev[:, b:b + 1], start=True, stop=True)
                    nc.tensor.matmul(ps_c[:, b:b + 1],
                                     lhsT=wt[:, b * 256 + 128: (b + 1) * 256],
                                     rhs=h_prev[:, b:b + 1], start=True, stop=True)

                cols = slice(t * 8, t * 8 + BPC)
                tu = spool.tile([128, BPC], F32, tag="tu")
                nc.vector.tensor_add(tu[:], ps_u[:], ukiT[:, cols])
                uu = spool.tile([128, BPC], F32, tag="uu")
                nc.scalar.activation(uu[:], tu[:], mybir.ActivationFunctionType.Sigmoid)
                tcn = spool.tile([128, BPC], F32, tag="tc")
                nc.vector.tensor_add(tcn[:], ps_c[:], ckiT[:, cols])
                cc = spool.tile([128, BPC], F32, tag="cc")
                nc.scalar.activation(cc[:], tcn[:], mybir.ActivationFunctionType.Tanh)
                dd = spool.tile([128, BPC], F32, tag="dd")
                nc.vector.tensor_sub(dd[:], h_prev, cc[:])
                ee = spool.tile([128, BPC], F32, tag="ee")
                nc.vector.tensor_mul(ee[:], uu[:], dd[:])
                nc.vector.tensor_add(statesT[:, (t + 1) * 8:(t + 1) * 8 + BPC],
                                     cc[:], ee[:])

                # emit final-matmul tiles as soon as their states are complete
                while m_emitted < n_mtiles and (m_emitted + 1) * 128 <= (t + 1) * 8:
                    emit_mtile(m_emitted)
                    m_emitted += 1
            while m_emitted < n_mtiles:
                emit_mtile(m_emitted)
                m_emitted += 1

    nc.compile()
    return nc


def prep_inputs(user_ids, item_ids, h0, gate_ku, gate_ki, gate_bias,
                cand_ku, cand_ki, cand_bias, ws, t_steps=T):
    """Host-side sharding/arrangement -> per-core in_maps."""
    ni = BPC * t_steps
    n_cb = (ni + 127) // 128
    wcomb = np.concatenate([gate_ku[:, :, H:], cand_ku], axis=2)
    wcomb = np.ascontiguousarray(
        wcomb.reshape((U + 1) * H, 2 * H).astype(NP_BF16))
    kicomb = np.ascontiguousarray(
        np.concatenate([gate_ki[:, H:], cand_ki], axis=1), np.float32)
    bias_u = np.ascontiguousarray(gate_bias[H:].reshape(H, 1), np.float32)
    bias_c = np.ascontiguousarray(cand_bias.reshape(H, 1), np.float32)
    ws_c = np.ascontiguousarray(np.asarray(ws, np.float32).astype(NP_BF16))

    in_maps = []
    for c in range(N_CORES):
        rows = slice(c * BPC, (c + 1) * BPC)
        # uidx[0, i] = uid_i*H : row offsets into the combined table, i = t*8+b
        uid_flat = np.ascontiguousarray(
            user_ids[rows, :t_steps], np.int32).T.reshape(-1)  # [ni]
        idxmat = (uid_flat * H).reshape(1, ni)
        iids_flat = np.ascontiguousarray(
            item_ids[rows, :t_steps], np.int32).T.reshape(-1)  # i = t*8+b
        iid_arr = np.zeros(n_cb * 128, np.int32)
        iid_arr[:ni] = iids_flat
        iid_arr = np.ascontiguousarray(iid_arr.reshape(n_cb, 128).T)
        h0t = np.ascontiguousarray(h0[rows].T.astype(NP_BF16))
        in_maps.append({
            "uidx": np.ascontiguousarray(idxmat), "iids": iid_arr, "h0t": h0t,
            "wcomb": wcomb, "kicomb": kicomb,
            "bias_u": bias_u, "bias_c": bias_c, "ws": ws_c,
        })
    return in_maps


def assemble_output(results, t_steps=T):
    ni = BPC * t_steps
    out = np.empty((B * t_steps, V), np.float32)
    for c in range(N_CORES):
        blk = np.asarray(results[c]["logits"])  # [ni, V] bf16, rows i = t*8+b
        out[c * ni:(c + 1) * ni] = (
            blk.reshape(t_steps, BPC, V).transpose(1, 0, 2)
            .reshape(ni, V).astype(np.float32))
    return out


_NC_CACHE = {}
USE_F32R = False  # retained for test.py compat; bf16 path is always used


def _get_nc(t_steps=T):
    if t_steps not in _NC_CACHE:
        _NC_CACHE[t_steps] = build_nc(t_steps)
    return _NC_CACHE[t_steps]


def kernel(user_ids, item_ids, h0, gate_ku, gate_ki, gate_bias,
           cand_ku, cand_ki, cand_bias, ws, trace=False):
    nc = _get_nc(T)
    in_maps = prep_inputs(np.asarray(user_ids), np.asarray(item_ids),
                          np.asarray(h0), np.asarray(gate_ku),
                          np.asarray(gate_ki), np.asarray(gate_bias),
                          np.asarray(cand_ku), np.asarray(cand_ki),
                          np.asarray(cand_bias), np.asarray(ws))
    res = bass_utils.run_bass_kernel_spmd(
        nc, in_maps, core_ids=list(range(N_CORES)), trace=trace)
    out = assemble_output(res.results)
    if trace:
        kernel.last_result = res
    return out
